# revision 14
# baseline (speedup 1.0000x reference)
"""Trainium2 Bass kernel for nn_AttentionPool_v1 (topk_masking).

Reference computation (N=16, C=64, W=512, H=512, RED=64, OUT_W=128):
    pooled = max(x, axes=(C, H))                  # [N, W]
    h      = pooled @ w1.T + b1                   # [N, RED]
    h      = BN1d(h, batch stats) -> relu         # [N, RED]
    att    = softmax(h @ w2.T + b2, axis=1)       # [N, W]
    idx    = sort(top_k(att, OUT_W).indices)      # [N, OUT_W]
    out    = x[n, :, idx[n], :]                   # [N, C, OUT_W, H]

Sharding: data-parallel over batch N across 8 cores (2 samples/core).
BatchNorm batch statistics are exact: ONE tiny AllGather of the local
hT columns (512 B/rank); every core then computes the identical
two-pass batch stats (reference formula) locally.

Device algorithm notes:
  * softmax is monotonic per-row => top-k on the logits z directly.
  * top-k via ranking: rank[i] = #{j: z[j] > z[i]} + #{j<i: z[j]==z[i]};
    keep = rank < OUT_W selects exactly the top-k set with jax.lax.top_k
    tie-breaking; output order is ascending index == jnp.sort(idx).
    The two counts are computed with single fused DVE ops
    (scalar_tensor_tensor / tensor_tensor_reduce with accum_out).
  * compaction: pos = (strict lower triangular) @ keep gives each kept
    index its output slot; one-hot(pos) matmul iota gives sorted idx
    values; indirect DMA row-gather moves the selected rows.
  * x is fed host-transposed as [n, w, c, h] so the (C,H) max-pool
    reads contiguous per-partition runs and the gather fetches
    contiguous (c, h) row slices with a single [128, 1] index column
    (multi-column indirect offsets hang the hardware DGE).
  * the device writes output as [n, OUT_W, C, H] (contiguous 16 KB
    runs per gathered row); the host transposes back to [n, C, OUT_W, H].
"""

import numpy as np

import concourse.bacc as bacc
import concourse.bass as bass
import concourse.mybir as mybir
import concourse.tile as tile
from concourse.bass import IndirectOffsetOnAxis
from concourse.bass_utils import run_bass_kernel_spmd

F32 = mybir.dt.float32
I32 = mybir.dt.int32
ALU = mybir.AluOpType
AX = mybir.AxisListType

# full-problem config
N_FULL, C_FULL, W_FULL, H_FULL = 16, 64, 512, 512
RED_FULL, OUTW_FULL = 64, 128
NCORES_FULL = 8
BN_EPS = 1e-5


class Cfg:
    def __init__(self, ncores=NCORES_FULL, n_loc=N_FULL // NCORES_FULL, c=C_FULL,
                 w=W_FULL, h=H_FULL, red=RED_FULL, out_w=OUTW_FULL,
                 chains=8, gw=16, cpt=8, gbufs=4):
        assert w % 128 == 0
        self.ncores, self.n_loc, self.c, self.w, self.h = ncores, n_loc, c, w, h
        self.red, self.out_w = red, out_w
        self.wch = w // 128
        self.chains = chains              # x staging buffers (phase 1)
        self.gw = gw                      # channels per indirect gather
        self.gbufs = gbufs                # gather staging buffers (phase 3)
        self.cpt = min(cpt, c)            # channels per phase-1 tile
        assert c % self.cpt == 0 and c % gw == 0
        assert out_w <= 128 and red <= 128


def kernel_body(tc, out_ap, ins, cfg: Cfg, upto=None, dbg=None):
    """Emit the kernel IR. `ins` is a dict name -> DRAM AP (see host_inputs).

    upto/dbg: debug early-exit -- after the named stage, DMA the probe
    tile into `dbg` (a [128, 64] f32 DRAM output) and stop emitting.
    """
    nc = tc.nc
    n_loc, c, w, h, red, out_w = (cfg.n_loc, cfg.c, cfg.w, cfg.h, cfg.red,
                                  cfg.out_w)
    wch, gw = cfg.wch, cfg.gw
    nbatch = cfg.ncores * n_loc
    xs = ins["xs"]
    group = [list(range(cfg.ncores))]

    dramp = tc.alloc_tile_pool(name="dram", bufs=1, space="DRAM")
    # dummy collective issued first: absorbs the ncfw first-collective
    # warmup (~20+ us) while phase 1 streams x, so the real AllGather
    # later runs at steady-state latency
    wu_i = dramp.tile([2, 2], F32)
    wu_o = dramp.tile([2, 2], F32)
    wup = tc.alloc_tile_pool(name="wup", bufs=1)
    wu_sb = wup.tile([2, 2], F32)
    nc.vector.memset(wu_sb[:], 0.0)
    nc.scalar.dma_start(wu_i[:], wu_sb[:])
    nc.gpsimd.collective_compute("AllReduce", ALU.add, replica_groups=group,
                                 ins=[wu_i.opt()], outs=[wu_o.opt()])
    wup.release()

    # constants on the ACT HWDGE ring so phase-1 x loads own the SP ring
    constp = tc.alloc_tile_pool(name="const", bufs=1)
    w1t_sb = constp.tile([128, wch * red], F32)
    nc.scalar.dma_start(w1t_sb[:], ins["w1t"])
    w2t_sb = constp.tile([red, w], F32)
    nc.scalar.dma_start(w2t_sb[:], ins["w2t"])
    b1_sb = constp.tile([red, 1], F32)
    nc.scalar.dma_start(b1_sb[:], ins["b1c"])
    gm_sb = constp.tile([red, 1], F32)
    nc.scalar.dma_start(gm_sb[:], ins["gmc"])
    bt_sb = constp.tile([red, 1], F32)
    nc.scalar.dma_start(bt_sb[:], ins["btc"])
    b2_sb = constp.tile([128, wch], F32)
    nc.scalar.dma_start(b2_sb[:], ins["b2t"])
    idn_sb = constp.tile([128, 128], F32)
    nc.scalar.dma_start(idn_sb[:], ins["idn"])
    ones_sb = constp.tile([128, 128], F32)
    nc.scalar.dma_start(ones_sb[:], ins["ones"])
    triu_sb = constp.tile([128, 128], F32)
    nc.scalar.dma_start(triu_sb[:], ins["triu"])
    irow_sb = constp.tile([128, out_w], F32)
    nc.scalar.dma_start(irow_sb[:], ins["irow"])
    icol_sb = constp.tile([128, wch], F32)
    nc.scalar.dma_start(icol_sb[:], ins["icol"])
    trim_sb = constp.tile([128, wch * w], F32)
    nc.scalar.dma_start(trim_sb[:], ins["trim"])

    mainp = tc.alloc_tile_pool(name="main", bufs=1)
    psum_small = tc.alloc_tile_pool(name="ps_small", bufs=2, space="PSUM")
    psum_t = tc.alloc_tile_pool(name="ps_t", bufs=2, space="PSUM")

    # ---------------- phase 1: pooled[w] = max over (c, h) --------------
    # xs layout is [n, w, c, h] (host-transposed), so each w-partition
    # reads a contiguous (c, h) run.  Loop k-major so the hT matmul
    # chunks chase the pooled chunks and the last matmul lands right
    # after the last tile.
    cpt = cfg.cpt                     # channels per tile
    nct = c // cpt
    pooledT = mainp.tile([128, wch * n_loc], F32)  # [p, k*n_loc+n]
    hT_ps = psum_small.tile([red, n_loc], F32, tag="mm")
    with tc.tile_pool(name="xp", bufs=cfg.chains) as xp, \
            tc.tile_pool(name="cmp", bufs=1) as cm_pool:
        colmax = []
        for n in range(n_loc):
            cm = cm_pool.tile([128, wch * nct], F32, name=f"colmax{n}",
                              tag=f"colmax{n}")
            colmax.append(cm)
        for k in range(wch):
            for n in range(n_loc):
                for t in range(nct):
                    xt = xp.tile([128, cpt * h], F32, tag="xt",
                                 name=f"xt{n}_{k}_{t}")
                    src = xs[n, k * 128:(k + 1) * 128, t * cpt:(t + 1) * cpt, :]
                    nc.sync.dma_start(
                        out=xt[:].rearrange("p (cc hh) -> p cc hh", cc=cpt),
                        in_=src)
                    nc.vector.reduce_max(
                        out=colmax[n][:, k * nct + t: k * nct + t + 1],
                        in_=xt[:], axis=AX.X)
                nc.vector.reduce_max(
                    out=pooledT[:, k * n_loc + n: k * n_loc + n + 1],
                    in_=colmax[n][:, k * nct:(k + 1) * nct], axis=AX.X)
            # hT[r, nl] += w1[r, w_k] pooled_k[nl]
            nc.tensor.matmul(out=hT_ps[:], lhsT=w1t_sb[:, k * red:(k + 1) * red],
                             rhs=pooledT[:, k * n_loc:(k + 1) * n_loc],
                             start=(k == 0), stop=(k == wch - 1))

    if upto == "pool":
        nc.sync.dma_start(dbg[:, :cfg.wch * n_loc], pooledT[:])
        for p in (psum_t, psum_small, dramp, mainp, constp):
            p.release()
        return

    # ---------------- phase 2: AllGather h, local BN stats --------------
    hT = mainp.tile([red, n_loc], F32)
    nc.vector.tensor_scalar_add(hT[:], hT_ps[:], b1_sb[:, :1])

    agi = dramp.tile([red, n_loc], F32)
    ago = dramp.tile([cfg.ncores * red, n_loc], F32)
    nc.sync.dma_start(agi[:], hT[:])
    nc.gpsimd.collective_compute("AllGather", ALU.bypass, replica_groups=group,
                                 ins=[agi.opt()], outs=[ago.opt()])
    h_all = mainp.tile([red, nbatch], F32)   # col = core*n_loc + nl = n
    nc.sync.dma_start(
        out=h_all[:].rearrange("r (cr nl) -> r cr nl", cr=cfg.ncores),
        in_=ago[:].rearrange("(cr r) nl -> cr r nl", cr=cfg.ncores
                             ).rearrange("cr r nl -> r cr nl"))

    if upto == "hall":
        nc.sync.dma_start(dbg[:red, :nbatch], h_all[:])
        for p in (psum_t, psum_small, dramp, mainp, constp):
            p.release()
        return

    # two-pass batch stats, reference formula, identical on every core
    ssum = mainp.tile([red, 1], F32)
    nc.vector.reduce_sum(out=ssum[:], in_=h_all[:], axis=AX.X)
    mu = mainp.tile([red, 1], F32)
    nc.vector.tensor_scalar_mul(mu[:], ssum[:], 1.0 / nbatch)
    cen_all = mainp.tile([red, nbatch], F32)
    nc.vector.tensor_scalar_sub(cen_all[:], h_all[:], mu[:, :1])
    sq_all = mainp.tile([red, nbatch], F32)
    nc.vector.tensor_mul(sq_all[:], cen_all[:], cen_all[:])
    vsum = mainp.tile([red, 1], F32)
    nc.vector.reduce_sum(out=vsum[:], in_=sq_all[:], axis=AX.X)

    # rstd = 1/sqrt(var + eps);  hr = relu(gamma*rstd*(hT-mu) + beta)
    ve = mainp.tile([red, 1], F32)
    nc.vector.tensor_scalar(ve[:], vsum[:], 1.0 / nbatch, BN_EPS,
                            op0=ALU.mult, op1=ALU.add)
    sd = mainp.tile([red, 1], F32)
    nc.scalar.sqrt(sd[:], ve[:])
    rstd = mainp.tile([red, 1], F32)
    nc.vector.reciprocal(rstd[:], sd[:])
    gs = mainp.tile([red, 1], F32)
    nc.vector.tensor_mul(gs[:], gm_sb[:], rstd[:])
    centered = mainp.tile([red, n_loc], F32)
    nc.vector.tensor_scalar_sub(centered[:], hT[:], mu[:, :1])
    hr = mainp.tile([red, n_loc], F32)
    nc.vector.tensor_scalar(hr[:], centered[:], gs[:, :1], bt_sb[:, :1],
                            op0=ALU.mult, op1=ALU.add)
    nc.vector.tensor_scalar_max(hr[:], hr[:], 0.0)

    if upto == "hr":
        nc.sync.dma_start(dbg[:red, :n_loc], hr[:])
        for p in (psum_t, psum_small, dramp, mainp, constp):
            p.release()
        return

    # zT[wq, nl] = w2 @ hr + b2   (logits, transposed)
    zT = mainp.tile([128, wch * n_loc], F32)  # col = q*n_loc + nl
    for q in range(wch):
        z_ps = psum_small.tile([128, n_loc], F32, tag="mm", name=f"z{q}")
        nc.tensor.matmul(out=z_ps[:], lhsT=w2t_sb[:, q * 128:(q + 1) * 128],
                         rhs=hr[:], start=True, stop=True)
        nc.vector.tensor_scalar_add(zT[:, q * n_loc:(q + 1) * n_loc], z_ps[:],
                                    b2_sb[:, q:q + 1])

    if upto == "z":
        nc.sync.dma_start(dbg[:, :cfg.wch * n_loc], zT[:])
        for p in (psum_t, psum_small, dramp, mainp, constp):
            p.release()
        return

    # --------- per-sample: ranking -> compaction -> gather --------------
    x_rows = xs.rearrange("n w c h -> (n w) (c h)")
    rank_nl, keep_nl, pos_nl, idx_i, zrows = [], [], [], [], []
    stage_n = {"zrow": 1, "rank": 2, "pos": 3, "idx": 4}.get(upto, 99)
    gp = tc.alloc_tile_pool(name="gp", bufs=cfg.gbufs)
    for nl in range(n_loc):
        # zrow[p, j] = z[nl, j] for all p (z replicated along partitions)
        zr = mainp.tile([128, w], F32, tag=f"zrow{nl}", name=f"zrow{nl}")
        zrows.append(zr)
        for q in range(wch):
            t_ps = psum_t.tile([128, 128], F32, tag="tp", name=f"tp{nl}_{q}")
            zcol = zT[:, q * n_loc + nl: q * n_loc + nl + 1]
            nc.tensor.transpose(out=t_ps[:], in_=zcol.to_broadcast([128, 128]),
                                identity=idn_sb[:])
            nc.scalar.copy(zr[:, q * 128:(q + 1) * 128], t_ps[:])
        if stage_n <= 1:
            continue

        tie = mainp.tile([128, wch], F32, name=f"tie{nl}")
        rank = mainp.tile([128, wch], F32, name=f"rank{nl}")
        keep = mainp.tile([128, wch], F32, name=f"keep{nl}")
        scr = mainp.tile([128, w], F32, tag=f"scr{nl}", name=f"scr{nl}")
        scr2 = mainp.tile([128, w], F32, tag=f"scr2_{nl}", name=f"scr2_{nl}")
        rank_nl.append(rank); keep_nl.append(keep)
        for q in range(wch):
            col = q * n_loc + nl
            zcol = zT[:, col: col + 1]
            zcb = zcol.to_broadcast([128, w])
            # tie[q] = #{j < i: z[j] == z[i]}   (i = q*128 + p)
            nc.vector.tensor_tensor(out=scr[:], in0=zr[:], in1=zcb,
                                    op=ALU.is_equal)
            nc.vector.tensor_mul(scr[:], scr[:],
                                 trim_sb[:, q * w:(q + 1) * w])
            nc.vector.reduce_sum(out=tie[:, q:q + 1], in_=scr[:], axis=AX.X)
            # rank[q] = tie[q] + #{j: z[j] > z[i]}
            nc.vector.tensor_tensor(out=scr2[:], in0=zr[:], in1=zcb,
                                    op=ALU.is_gt)
            nc.vector.reduce_sum(out=rank[:, q:q + 1], in_=scr2[:], axis=AX.X)
            nc.vector.tensor_add(rank[:, q:q + 1], rank[:, q:q + 1],
                                 tie[:, q:q + 1])
            nc.vector.tensor_scalar(keep[:, q:q + 1], rank[:, q:q + 1],
                                    float(out_w), None, op0=ALU.is_lt)
        if stage_n <= 2:
            continue

        # pos[i] = #{j < i: keep[j]} via triangular matmuls
        pos = mainp.tile([128, wch], F32, name=f"pos{nl}")
        pos_nl.append(pos)
        for m in range(wch):
            pos_ps = psum_small.tile([128, 1], F32, tag="mm",
                                     name=f"pos{nl}_{m}")
            for q in range(m + 1):
                nc.tensor.matmul(out=pos_ps[:],
                                 lhsT=(triu_sb[:] if q == m else ones_sb[:]),
                                 rhs=keep[:, q:q + 1],
                                 start=(q == 0), stop=(q == m))
            nc.scalar.copy(pos[:, m:m + 1], pos_ps[:])
        if stage_n <= 3:
            continue

        # one-hot [i, slot]; idx[slot] = sum_i onehot * i ; then gather
        oh = []
        for q in range(wch):
            o = mainp.tile([128, out_w], F32, tag=f"oh{nl}_{q}",
                           name=f"oh{nl}_{q}")
            nc.vector.tensor_scalar(o[:], irow_sb[:], pos[:, q:q + 1],
                                    keep[:, q:q + 1],
                                    op0=ALU.is_equal, op1=ALU.mult)
            oh.append(o)
        idx_ps = psum_small.tile([out_w, 1], F32, tag="mm", name=f"idx{nl}")
        for q in range(wch):
            nc.tensor.matmul(out=idx_ps[:], lhsT=oh[q][:],
                             rhs=icol_sb[:, q:q + 1],
                             start=(q == 0), stop=(q == wch - 1))
        # row index in the flat [n*w, c*h] view = nl*w + idx
        idx_f = mainp.tile([out_w, 1], F32, name=f"idxf{nl}")
        nc.vector.tensor_single_scalar(idx_f[:], idx_ps[:],
                                       float(nl * w), ALU.add)
        ii = mainp.tile([out_w, 1], I32, name=f"idxi{nl}")
        nc.vector.tensor_copy(ii[:], idx_f[:])
        idx_i.append((ii, idx_f))

        if stage_n <= 4:
            continue
        # gather: each index fetches a contiguous gw*h slice of the
        # (c, h) row; output written as [out_w, c, h] (host transposes)
        for q2 in range(c // gw):
            gt = gp.tile([out_w, gw * h], F32, tag="gt",
                         name=f"gt{nl}_{q2}")
            nc.gpsimd.indirect_dma_start(
                out=gt[:], out_offset=None, in_=x_rows,
                in_offset=IndirectOffsetOnAxis(ap=ii[:out_w, :1], axis=0),
                element_offset=q2 * gw * h)
            dst = out_ap[nl, :, q2 * gw:(q2 + 1) * gw, :]
            nc.sync.dma_start(out=dst,
                              in_=gt[:].rearrange("j (i h) -> j i h", i=gw))

    if upto == "zrow":
        nc.sync.dma_start(dbg[:, 0:32], zrows[0][:, 0:32])
        nc.sync.dma_start(dbg[:, 32:64], zrows[1][:, 0:32])
    if upto == "rank":
        nc.sync.dma_start(dbg[:, 0:wch], rank_nl[0][:])
        nc.sync.dma_start(dbg[:, 8:8 + wch], keep_nl[0][:])
        nc.sync.dma_start(dbg[:, 16:16 + wch], rank_nl[1][:])
        nc.sync.dma_start(dbg[:, 24:24 + wch], keep_nl[1][:])
    if upto == "pos":
        nc.sync.dma_start(dbg[:, 0:wch], pos_nl[0][:])
        nc.sync.dma_start(dbg[:, 8:8 + wch], pos_nl[1][:])
    if upto == "idx":
        nc.sync.dma_start(dbg[:out_w, 0:1], idx_i[0][1][:])
        nc.sync.dma_start(dbg[:out_w, 1:2], idx_i[1][1][:])

    gp.release()
    psum_t.release()
    psum_small.release()
    dramp.release()
    mainp.release()
    constp.release()


def host_inputs(w1, b1, gamma, beta, w2, b2, cfg: Cfg):
    """Shared (non-sharded) input tensors, prepacked for the kernel."""
    c, w, red, out_w, wch = cfg.c, cfg.w, cfg.red, cfg.out_w, cfg.wch
    f = np.float32
    w1t = np.ascontiguousarray(
        w1.T.reshape(wch, 128, red).transpose(1, 0, 2).reshape(128, wch * red)
    ).astype(f)
    w2t = np.ascontiguousarray(w2.T).astype(f)
    b2t = np.ascontiguousarray(b2.reshape(wch, 128).T).astype(f)
    irow = np.tile(np.arange(out_w, dtype=f), (128, 1))
    icol = (np.arange(wch, dtype=f)[None, :] * 128
            + np.arange(128, dtype=f)[:, None])
    jj = np.arange(w, dtype=np.int64)[None, None, :]
    ii = (np.arange(wch, dtype=np.int64)[:, None, None] * 128
          + np.arange(128, dtype=np.int64)[None, :, None])
    trim = (jj < ii).astype(f).transpose(1, 0, 2).reshape(128, wch * w)
    return {
        "w1t": w1t,
        "w2t": w2t,
        "b1c": np.ascontiguousarray(b1.reshape(red, 1)).astype(f),
        "gmc": np.ascontiguousarray(gamma.reshape(red, 1)).astype(f),
        "btc": np.ascontiguousarray(beta.reshape(red, 1)).astype(f),
        "b2t": b2t,
        "idn": np.eye(128, dtype=f),
        "ones": np.ones((128, 128), dtype=f),
        "triu": np.triu(np.ones((128, 128), dtype=f), k=1),
        "irow": irow,
        "icol": np.ascontiguousarray(icol),
        "trim": np.ascontiguousarray(trim),
    }


def build_nc(cfg: Cfg, upto=None):
    nc = bacc.Bacc("TRN2", target_bir_lowering=False, debug=False,
                   num_devices=cfg.ncores)
    n_loc, c, w, h, red, out_w, wch = (cfg.n_loc, cfg.c, cfg.w, cfg.h,
                                       cfg.red, cfg.out_w, cfg.wch)
    ins = {}
    ins["xs"] = nc.dram_tensor("xs", [n_loc, w, c, h], F32,
                               kind="ExternalInput").ap()
    for name, shape, dt in [
        ("w1t", [128, wch * red], F32),
        ("w2t", [red, w], F32),
        ("b1c", [red, 1], F32),
        ("gmc", [red, 1], F32),
        ("btc", [red, 1], F32),
        ("b2t", [128, wch], F32),
        ("idn", [128, 128], F32),
        ("ones", [128, 128], F32),
        ("triu", [128, 128], F32),
        ("irow", [128, out_w], F32),
        ("icol", [128, wch], F32),
        ("trim", [128, wch * w], F32),
    ]:
        ins[name] = nc.dram_tensor(name, shape, dt, kind="ExternalInput").ap()
    out = nc.dram_tensor("out", [n_loc, out_w, c, h], F32,
                         kind="ExternalOutput").ap()
    dbg = None
    if upto is not None:
        dbg = nc.dram_tensor("dbg", [128, 64], F32,
                             kind="ExternalOutput").ap()
    with tile.TileContext(nc) as tc:
        kernel_body(tc, out, ins, cfg, upto=upto, dbg=dbg)
    nc.compile()
    return nc


_CACHE = {}


def get_nc(cfg=None):
    cfg = cfg or Cfg()
    if "nc" not in _CACHE:
        _CACHE["nc"] = build_nc(cfg)
    return _CACHE["nc"]


def make_in_maps(inputs, cfg=None):
    cfg = cfg or Cfg()
    x = np.ascontiguousarray(np.asarray(inputs["x"], dtype=np.float32))
    shared = host_inputs(np.asarray(inputs["w1"]), np.asarray(inputs["b1"]),
                         np.asarray(inputs["gamma"]),
                         np.asarray(inputs["beta"]),
                         np.asarray(inputs["w2"]), np.asarray(inputs["b2"]),
                         cfg)
    in_maps = []
    for i in range(cfg.ncores):
        m = dict(shared)
        m["xs"] = np.ascontiguousarray(
            x[i * cfg.n_loc:(i + 1) * cfg.n_loc].transpose(0, 2, 1, 3))
        in_maps.append(m)
    return in_maps


def kernel(**inputs):
    cfg = Cfg()
    nc = get_nc(cfg)
    in_maps = make_in_maps(inputs, cfg)
    res = run_bass_kernel_spmd(nc, in_maps, list(range(cfg.ncores)))
    # device output layout is [n_loc, OUT_W, C, H]; reference wants
    # [n, C, OUT_W, H]
    outs = [np.ascontiguousarray(r["out"].transpose(0, 2, 1, 3))
            for r in res.results]
    return np.concatenate(outs, axis=0)


# revision 16
# speedup vs baseline: 1.0030x; 1.0030x over previous
"""Trainium2 Bass kernel for nn_AttentionPool_v1 (topk_masking).

Reference computation (N=16, C=64, W=512, H=512, RED=64, OUT_W=128):
    pooled = max(x, axes=(C, H))                  # [N, W]
    h      = pooled @ w1.T + b1                   # [N, RED]
    h      = BN1d(h, batch stats) -> relu         # [N, RED]
    att    = softmax(h @ w2.T + b2, axis=1)       # [N, W]
    idx    = sort(top_k(att, OUT_W).indices)      # [N, OUT_W]
    out    = x[n, :, idx[n], :]                   # [N, C, OUT_W, H]

Sharding: data-parallel over batch N across 8 cores (2 samples/core).
BatchNorm batch statistics are exact: ONE tiny AllGather of the local
hT columns (512 B/rank); every core then computes the identical
two-pass batch stats (reference formula) locally.

Device algorithm notes:
  * softmax is monotonic per-row => top-k on the logits z directly.
  * top-k via ranking: rank[i] = #{j: z[j] > z[i]} + #{j<i: z[j]==z[i]};
    keep = rank < OUT_W selects exactly the top-k set with jax.lax.top_k
    tie-breaking; output order is ascending index == jnp.sort(idx).
    Each sample is ranked, compacted and gathered as its own dependency
    chain so sample 0's gather overlaps sample 1's ranking.
  * compaction: pos = (strict lower triangular) @ keep gives each kept
    index its output slot; one-hot(pos) matmul iota gives sorted idx
    values; indirect DMA row-gather moves the selected rows.
  * x is fed host-transposed as [n, w, c, h] so the (C,H) max-pool
    reads contiguous per-partition runs and the gather fetches
    contiguous (c, h) row slices with a single [128, 1] index column
    (multi-column indirect offsets hang the hardware DGE).
  * the device writes output as [n, OUT_W, C, H] (contiguous 16 KB
    runs per gathered row); the host transposes back to [n, C, OUT_W, H].
"""

import numpy as np

import concourse.bacc as bacc
import concourse.bass as bass
import concourse.mybir as mybir
import concourse.tile as tile
from concourse.bass import IndirectOffsetOnAxis
from concourse.bass_utils import run_bass_kernel_spmd

F32 = mybir.dt.float32
I32 = mybir.dt.int32
ALU = mybir.AluOpType
AX = mybir.AxisListType

# full-problem config
N_FULL, C_FULL, W_FULL, H_FULL = 16, 64, 512, 512
RED_FULL, OUTW_FULL = 64, 128
NCORES_FULL = 8
BN_EPS = 1e-5


class Cfg:
    def __init__(self, ncores=NCORES_FULL, n_loc=N_FULL // NCORES_FULL, c=C_FULL,
                 w=W_FULL, h=H_FULL, red=RED_FULL, out_w=OUTW_FULL,
                 chains=8, gw=8, cpt=8, gbufs=6):
        assert w % 128 == 0
        self.ncores, self.n_loc, self.c, self.w, self.h = ncores, n_loc, c, w, h
        self.red, self.out_w = red, out_w
        self.wch = w // 128
        self.chains = chains              # x staging buffers (phase 1)
        self.gw = gw                      # channels per indirect gather
        self.gbufs = gbufs                # gather staging buffers (phase 3)
        self.cpt = min(cpt, c)            # channels per phase-1 tile
        assert c % self.cpt == 0 and c % gw == 0
        assert out_w <= 128 and red <= 128


def kernel_body(tc, out_ap, ins, cfg: Cfg, upto=None, dbg=None):
    """Emit the kernel IR. `ins` is a dict name -> DRAM AP (see host_inputs).

    upto/dbg: debug early-exit -- after the named stage, DMA the probe
    tile into `dbg` (a [128, 64] f32 DRAM output) and stop emitting.
    """
    nc = tc.nc
    n_loc, c, w, h, red, out_w = (cfg.n_loc, cfg.c, cfg.w, cfg.h, cfg.red,
                                  cfg.out_w)
    wch, gw = cfg.wch, cfg.gw
    nbatch = cfg.ncores * n_loc
    xs = ins["xs"]
    group = [list(range(cfg.ncores))]

    dramp = tc.alloc_tile_pool(name="dram", bufs=1, space="DRAM")
    # dummy collective issued first: absorbs the ncfw first-collective
    # warmup (~20+ us) while phase 1 streams x, so the real AllGather
    # later runs at steady-state latency
    wu_i = dramp.tile([2, 2], F32)
    wu_o = dramp.tile([2, 2], F32)
    wup = tc.alloc_tile_pool(name="wup", bufs=1)
    wu_sb = wup.tile([2, 2], F32)
    nc.vector.memset(wu_sb[:], 0.0)
    nc.scalar.dma_start(wu_i[:], wu_sb[:])
    nc.gpsimd.collective_compute("AllReduce", ALU.add, replica_groups=group,
                                 ins=[wu_i.opt()], outs=[wu_o.opt()])
    wup.release()

    # constants on the ACT HWDGE ring so phase-1 x loads own the SP ring
    constp = tc.alloc_tile_pool(name="const", bufs=1)
    w1t_sb = constp.tile([128, wch * red], F32)
    nc.scalar.dma_start(w1t_sb[:], ins["w1t"])
    w2t_sb = constp.tile([red, w], F32)
    nc.scalar.dma_start(w2t_sb[:], ins["w2t"])
    b1_sb = constp.tile([red, 1], F32)
    nc.scalar.dma_start(b1_sb[:], ins["b1c"])
    gm_sb = constp.tile([red, 1], F32)
    nc.scalar.dma_start(gm_sb[:], ins["gmc"])
    bt_sb = constp.tile([red, 1], F32)
    nc.scalar.dma_start(bt_sb[:], ins["btc"])
    b2_sb = constp.tile([128, wch], F32)
    nc.scalar.dma_start(b2_sb[:], ins["b2t"])
    idn_sb = constp.tile([128, 128], F32)
    nc.scalar.dma_start(idn_sb[:], ins["idn"])
    ones_sb = constp.tile([128, 128], F32)
    nc.scalar.dma_start(ones_sb[:], ins["ones"])
    triu_sb = constp.tile([128, 128], F32)
    nc.scalar.dma_start(triu_sb[:], ins["triu"])
    irow_sb = constp.tile([128, out_w], F32)
    nc.scalar.dma_start(irow_sb[:], ins["irow"])
    icol_sb = constp.tile([128, wch], F32)
    nc.scalar.dma_start(icol_sb[:], ins["icol"])
    trim_sb = constp.tile([128, wch * w], F32)
    nc.scalar.dma_start(trim_sb[:], ins["trim"])

    mainp = tc.alloc_tile_pool(name="main", bufs=1)
    psum_small = tc.alloc_tile_pool(name="ps_small", bufs=2, space="PSUM")
    psum_t = tc.alloc_tile_pool(name="ps_t", bufs=2, space="PSUM")

    # ---------------- phase 1: pooled[w] = max over (c, h) --------------
    # xs layout is [n, w, c, h] (host-transposed), so each w-partition
    # reads a contiguous (c, h) run.  Loop k-major so the hT matmul
    # chunks chase the pooled chunks and the last matmul lands right
    # after the last tile.
    cpt = cfg.cpt                     # channels per tile
    nct = c // cpt
    pooledT = mainp.tile([128, wch * n_loc], F32)  # [p, k*n_loc+n]
    hT_ps = psum_small.tile([red, n_loc], F32, tag="mm")
    with tc.tile_pool(name="xp", bufs=cfg.chains) as xp, \
            tc.tile_pool(name="cmp", bufs=1) as cm_pool:
        colmax = []
        for n in range(n_loc):
            cm = cm_pool.tile([128, wch * nct], F32, name=f"colmax{n}",
                              tag=f"colmax{n}")
            colmax.append(cm)
        for k in range(wch):
            for n in range(n_loc):
                for t in range(nct):
                    xt = xp.tile([128, cpt * h], F32, tag="xt",
                                 name=f"xt{n}_{k}_{t}")
                    src = xs[n, k * 128:(k + 1) * 128, t * cpt:(t + 1) * cpt, :]
                    nc.sync.dma_start(
                        out=xt[:].rearrange("p (cc hh) -> p cc hh", cc=cpt),
                        in_=src)
                    nc.vector.reduce_max(
                        out=colmax[n][:, k * nct + t: k * nct + t + 1],
                        in_=xt[:], axis=AX.X)
                nc.vector.reduce_max(
                    out=pooledT[:, k * n_loc + n: k * n_loc + n + 1],
                    in_=colmax[n][:, k * nct:(k + 1) * nct], axis=AX.X)
            # hT[r, nl] += w1[r, w_k] pooled_k[nl]
            nc.tensor.matmul(out=hT_ps[:], lhsT=w1t_sb[:, k * red:(k + 1) * red],
                             rhs=pooledT[:, k * n_loc:(k + 1) * n_loc],
                             start=(k == 0), stop=(k == wch - 1))

    if upto == "pool":
        nc.sync.dma_start(dbg[:, :cfg.wch * n_loc], pooledT[:])
        for p in (psum_t, psum_small, dramp, mainp, constp):
            p.release()
        return

    # ---------------- phase 2: AllGather h, local BN stats --------------
    hT = mainp.tile([red, n_loc], F32)
    nc.vector.tensor_scalar_add(hT[:], hT_ps[:], b1_sb[:, :1])

    agi = dramp.tile([red, n_loc], F32)
    ago = dramp.tile([cfg.ncores * red, n_loc], F32)
    nc.sync.dma_start(agi[:], hT[:])
    nc.gpsimd.collective_compute("AllGather", ALU.bypass, replica_groups=group,
                                 ins=[agi.opt()], outs=[ago.opt()])
    h_all = mainp.tile([red, nbatch], F32)   # col = core*n_loc + nl = n
    nc.sync.dma_start(
        out=h_all[:].rearrange("r (cr nl) -> r cr nl", cr=cfg.ncores),
        in_=ago[:].rearrange("(cr r) nl -> cr r nl", cr=cfg.ncores
                             ).rearrange("cr r nl -> r cr nl"))

    if upto == "hall":
        nc.sync.dma_start(dbg[:red, :nbatch], h_all[:])
        for p in (psum_t, psum_small, dramp, mainp, constp):
            p.release()
        return

    # two-pass batch stats, reference formula, identical on every core
    ssum = mainp.tile([red, 1], F32)
    nc.vector.reduce_sum(out=ssum[:], in_=h_all[:], axis=AX.X)
    mu = mainp.tile([red, 1], F32)
    nc.vector.tensor_scalar_mul(mu[:], ssum[:], 1.0 / nbatch)
    cen_all = mainp.tile([red, nbatch], F32)
    nc.vector.tensor_scalar_sub(cen_all[:], h_all[:], mu[:, :1])
    sq_all = mainp.tile([red, nbatch], F32)
    nc.vector.tensor_mul(sq_all[:], cen_all[:], cen_all[:])
    vsum = mainp.tile([red, 1], F32)
    nc.vector.reduce_sum(out=vsum[:], in_=sq_all[:], axis=AX.X)

    # rstd = 1/sqrt(var + eps);  hr = relu(gamma*rstd*(hT-mu) + beta)
    ve = mainp.tile([red, 1], F32)
    nc.vector.tensor_scalar(ve[:], vsum[:], 1.0 / nbatch, BN_EPS,
                            op0=ALU.mult, op1=ALU.add)
    sd = mainp.tile([red, 1], F32)
    nc.scalar.sqrt(sd[:], ve[:])
    rstd = mainp.tile([red, 1], F32)
    nc.vector.reciprocal(rstd[:], sd[:])
    gs = mainp.tile([red, 1], F32)
    nc.vector.tensor_mul(gs[:], gm_sb[:], rstd[:])
    centered = mainp.tile([red, n_loc], F32)
    nc.vector.tensor_scalar_sub(centered[:], hT[:], mu[:, :1])
    hr = mainp.tile([red, n_loc], F32)
    nc.vector.tensor_scalar(hr[:], centered[:], gs[:, :1], bt_sb[:, :1],
                            op0=ALU.mult, op1=ALU.add)
    nc.vector.tensor_scalar_max(hr[:], hr[:], 0.0)

    if upto == "hr":
        nc.sync.dma_start(dbg[:red, :n_loc], hr[:])
        for p in (psum_t, psum_small, dramp, mainp, constp):
            p.release()
        return

    # zT[wq, nl] = w2 @ hr + b2   (logits, transposed)
    zT = mainp.tile([128, wch * n_loc], F32)  # col = q*n_loc + nl
    for q in range(wch):
        z_ps = psum_small.tile([128, n_loc], F32, tag="mm", name=f"z{q}")
        nc.tensor.matmul(out=z_ps[:], lhsT=w2t_sb[:, q * 128:(q + 1) * 128],
                         rhs=hr[:], start=True, stop=True)
        nc.vector.tensor_scalar_add(zT[:, q * n_loc:(q + 1) * n_loc], z_ps[:],
                                    b2_sb[:, q:q + 1])

    if upto == "z":
        nc.sync.dma_start(dbg[:, :cfg.wch * n_loc], zT[:])
        for p in (psum_t, psum_small, dramp, mainp, constp):
            p.release()
        return

    # --------- per-sample: ranking -> compaction -> gather --------------
    x_rows = xs.rearrange("n w c h -> (n w) (c h)")
    rank_nl, keep_nl, pos_nl, idx_i, zrows = [], [], [], [], []
    stage_n = {"zrow": 1, "rank": 2, "pos": 3, "idx": 4}.get(upto, 99)
    gp = tc.alloc_tile_pool(name="gp", bufs=cfg.gbufs)
    for nl in range(n_loc):
        # zrow[p, j] = z[nl, j] for all p (z replicated along partitions)
        zr = mainp.tile([128, w], F32, tag=f"zrow{nl}", name=f"zrow{nl}")
        zrows.append(zr)
        for q in range(wch):
            t_ps = psum_t.tile([128, 128], F32, tag="tp", name=f"tp{nl}_{q}")
            zcol = zT[:, q * n_loc + nl: q * n_loc + nl + 1]
            nc.tensor.transpose(out=t_ps[:], in_=zcol.to_broadcast([128, 128]),
                                identity=idn_sb[:])
            nc.scalar.copy(zr[:, q * 128:(q + 1) * 128], t_ps[:])
        if stage_n <= 1:
            continue

        tie = mainp.tile([128, wch], F32, name=f"tie{nl}")
        rank = mainp.tile([128, wch], F32, name=f"rank{nl}")
        keep = mainp.tile([128, wch], F32, name=f"keep{nl}")
        scr = mainp.tile([128, w], F32, tag=f"scr{nl}", name=f"scr{nl}")
        scr2 = mainp.tile([128, w], F32, tag=f"scr2_{nl}", name=f"scr2_{nl}")
        rank_nl.append(rank); keep_nl.append(keep)
        for q in range(wch):
            col = q * n_loc + nl
            zcol = zT[:, col: col + 1]
            zcb = zcol.to_broadcast([128, w])
            # tie[q] = #{j < i: z[j] == z[i]}   (i = q*128 + p)
            nc.vector.tensor_tensor(out=scr[:], in0=zr[:], in1=zcb,
                                    op=ALU.is_equal)
            nc.vector.tensor_mul(scr[:], scr[:],
                                 trim_sb[:, q * w:(q + 1) * w])
            nc.vector.reduce_sum(out=tie[:, q:q + 1], in_=scr[:], axis=AX.X)
            # rank[q] = tie[q] + #{j: z[j] > z[i]}
            nc.vector.tensor_tensor(out=scr2[:], in0=zr[:], in1=zcb,
                                    op=ALU.is_gt)
            nc.vector.reduce_sum(out=rank[:, q:q + 1], in_=scr2[:], axis=AX.X)
            nc.vector.tensor_add(rank[:, q:q + 1], rank[:, q:q + 1],
                                 tie[:, q:q + 1])
            nc.vector.tensor_scalar(keep[:, q:q + 1], rank[:, q:q + 1],
                                    float(out_w), None, op0=ALU.is_lt)
        if stage_n <= 2:
            continue

        # pos[i] = #{j < i: keep[j]} via triangular matmuls
        pos = mainp.tile([128, wch], F32, name=f"pos{nl}")
        pos_nl.append(pos)
        for m in range(wch):
            pos_ps = psum_small.tile([128, 1], F32, tag="mm",
                                     name=f"pos{nl}_{m}")
            for q in range(m + 1):
                nc.tensor.matmul(out=pos_ps[:],
                                 lhsT=(triu_sb[:] if q == m else ones_sb[:]),
                                 rhs=keep[:, q:q + 1],
                                 start=(q == 0), stop=(q == m))
            nc.scalar.copy(pos[:, m:m + 1], pos_ps[:])
        if stage_n <= 3:
            continue

        # one-hot [i, slot]; idx[slot] = sum_i onehot * i ; then gather
        oh = []
        for q in range(wch):
            o = mainp.tile([128, out_w], F32, tag=f"oh{nl}_{q}",
                           name=f"oh{nl}_{q}")
            nc.vector.tensor_scalar(o[:], irow_sb[:], pos[:, q:q + 1],
                                    keep[:, q:q + 1],
                                    op0=ALU.is_equal, op1=ALU.mult)
            oh.append(o)
        idx_ps = psum_small.tile([out_w, 1], F32, tag="mm", name=f"idx{nl}")
        for q in range(wch):
            nc.tensor.matmul(out=idx_ps[:], lhsT=oh[q][:],
                             rhs=icol_sb[:, q:q + 1],
                             start=(q == 0), stop=(q == wch - 1))
        # row index in the flat [n*w, c*h] view = nl*w + idx
        idx_f = mainp.tile([out_w, 1], F32, name=f"idxf{nl}")
        nc.vector.tensor_single_scalar(idx_f[:], idx_ps[:],
                                       float(nl * w), ALU.add)
        ii = mainp.tile([out_w, 1], I32, name=f"idxi{nl}")
        nc.vector.tensor_copy(ii[:], idx_f[:])
        idx_i.append((ii, idx_f))

        if stage_n <= 4:
            continue
        # gather: each index fetches a contiguous gw*h slice of the
        # (c, h) row; output written as [out_w, c, h] (host transposes)
        for q2 in range(c // gw):
            gt = gp.tile([out_w, gw * h], F32, tag="gt",
                         name=f"gt{nl}_{q2}")
            nc.gpsimd.indirect_dma_start(
                out=gt[:], out_offset=None, in_=x_rows,
                in_offset=IndirectOffsetOnAxis(ap=ii[:out_w, :1], axis=0),
                element_offset=q2 * gw * h)
            dst = out_ap[nl, :, q2 * gw:(q2 + 1) * gw, :]
            nc.sync.dma_start(out=dst,
                              in_=gt[:].rearrange("j (i h) -> j i h", i=gw))

    if upto == "zrow":
        nc.sync.dma_start(dbg[:, 0:32], zrows[0][:, 0:32])
        nc.sync.dma_start(dbg[:, 32:64], zrows[1][:, 0:32])
    if upto == "rank":
        nc.sync.dma_start(dbg[:, 0:wch], rank_nl[0][:])
        nc.sync.dma_start(dbg[:, 8:8 + wch], keep_nl[0][:])
        nc.sync.dma_start(dbg[:, 16:16 + wch], rank_nl[1][:])
        nc.sync.dma_start(dbg[:, 24:24 + wch], keep_nl[1][:])
    if upto == "pos":
        nc.sync.dma_start(dbg[:, 0:wch], pos_nl[0][:])
        nc.sync.dma_start(dbg[:, 8:8 + wch], pos_nl[1][:])
    if upto == "idx":
        nc.sync.dma_start(dbg[:out_w, 0:1], idx_i[0][1][:])
        nc.sync.dma_start(dbg[:out_w, 1:2], idx_i[1][1][:])

    gp.release()
    psum_t.release()
    psum_small.release()
    dramp.release()
    mainp.release()
    constp.release()


def host_inputs(w1, b1, gamma, beta, w2, b2, cfg: Cfg):
    """Shared (non-sharded) input tensors, prepacked for the kernel."""
    c, w, red, out_w, wch = cfg.c, cfg.w, cfg.red, cfg.out_w, cfg.wch
    f = np.float32
    w1t = np.ascontiguousarray(
        w1.T.reshape(wch, 128, red).transpose(1, 0, 2).reshape(128, wch * red)
    ).astype(f)
    w2t = np.ascontiguousarray(w2.T).astype(f)
    b2t = np.ascontiguousarray(b2.reshape(wch, 128).T).astype(f)
    irow = np.tile(np.arange(out_w, dtype=f), (128, 1))
    icol = (np.arange(wch, dtype=f)[None, :] * 128
            + np.arange(128, dtype=f)[:, None])
    jj = np.arange(w, dtype=np.int64)[None, None, :]
    ii = (np.arange(wch, dtype=np.int64)[:, None, None] * 128
          + np.arange(128, dtype=np.int64)[None, :, None])
    trim = (jj < ii).astype(f).transpose(1, 0, 2).reshape(128, wch * w)
    return {
        "w1t": w1t,
        "w2t": w2t,
        "b1c": np.ascontiguousarray(b1.reshape(red, 1)).astype(f),
        "gmc": np.ascontiguousarray(gamma.reshape(red, 1)).astype(f),
        "btc": np.ascontiguousarray(beta.reshape(red, 1)).astype(f),
        "b2t": b2t,
        "idn": np.eye(128, dtype=f),
        "ones": np.ones((128, 128), dtype=f),
        "triu": np.triu(np.ones((128, 128), dtype=f), k=1),
        "irow": irow,
        "icol": np.ascontiguousarray(icol),
        "trim": np.ascontiguousarray(trim),
    }


def build_nc(cfg: Cfg, upto=None):
    nc = bacc.Bacc("TRN2", target_bir_lowering=False, debug=False,
                   num_devices=cfg.ncores)
    n_loc, c, w, h, red, out_w, wch = (cfg.n_loc, cfg.c, cfg.w, cfg.h,
                                       cfg.red, cfg.out_w, cfg.wch)
    ins = {}
    ins["xs"] = nc.dram_tensor("xs", [n_loc, w, c, h], F32,
                               kind="ExternalInput").ap()
    for name, shape, dt in [
        ("w1t", [128, wch * red], F32),
        ("w2t", [red, w], F32),
        ("b1c", [red, 1], F32),
        ("gmc", [red, 1], F32),
        ("btc", [red, 1], F32),
        ("b2t", [128, wch], F32),
        ("idn", [128, 128], F32),
        ("ones", [128, 128], F32),
        ("triu", [128, 128], F32),
        ("irow", [128, out_w], F32),
        ("icol", [128, wch], F32),
        ("trim", [128, wch * w], F32),
    ]:
        ins[name] = nc.dram_tensor(name, shape, dt, kind="ExternalInput").ap()
    out = nc.dram_tensor("out", [n_loc, out_w, c, h], F32,
                         kind="ExternalOutput").ap()
    dbg = None
    if upto is not None:
        dbg = nc.dram_tensor("dbg", [128, 64], F32,
                             kind="ExternalOutput").ap()
    with tile.TileContext(nc) as tc:
        kernel_body(tc, out, ins, cfg, upto=upto, dbg=dbg)
    nc.compile()
    return nc


_CACHE = {}


def get_nc(cfg=None):
    cfg = cfg or Cfg()
    if "nc" not in _CACHE:
        _CACHE["nc"] = build_nc(cfg)
    return _CACHE["nc"]


def make_in_maps(inputs, cfg=None):
    cfg = cfg or Cfg()
    x = np.ascontiguousarray(np.asarray(inputs["x"], dtype=np.float32))
    shared = host_inputs(np.asarray(inputs["w1"]), np.asarray(inputs["b1"]),
                         np.asarray(inputs["gamma"]),
                         np.asarray(inputs["beta"]),
                         np.asarray(inputs["w2"]), np.asarray(inputs["b2"]),
                         cfg)
    in_maps = []
    for i in range(cfg.ncores):
        m = dict(shared)
        m["xs"] = np.ascontiguousarray(
            x[i * cfg.n_loc:(i + 1) * cfg.n_loc].transpose(0, 2, 1, 3))
        in_maps.append(m)
    return in_maps


def kernel(**inputs):
    cfg = Cfg()
    nc = get_nc(cfg)
    in_maps = make_in_maps(inputs, cfg)
    res = run_bass_kernel_spmd(nc, in_maps, list(range(cfg.ncores)))
    # device output layout is [n_loc, OUT_W, C, H]; reference wants
    # [n, C, OUT_W, H]
    outs = [np.ascontiguousarray(r["out"].transpose(0, 2, 1, 3))
            for r in res.results]
    return np.concatenate(outs, axis=0)
